# revision 48
# baseline (speedup 1.0000x reference)
"""Trainium2 Bass kernel for DeepKernelNN GNN message passing (NNConv-style).

Strategy (8 NeuronCores, SPMD):
  - Host: sort edges by dst, shard contiguous 512-node dst ranges per core,
    pad each core to a common edge count. Precompute h0 = x@fc1+b (tiny),
    per-edge metadata (src idx, local dst, 1/deg), and augmented weights.
  - Device per layer: edge MLP feature-major (weights stationary on PE,
    fp32r 1 cyc/row), We = e2@kw3 edge-major via activation-stationary
    matmuls into PSUM, per-edge matvec msg = h[src] . We on DVE with
    per-partition-scalar FMAs, segment-sum via one-hot scatter matmul
    (S built on device from iota/is_equal, inv_deg folded in), NNConv
    update feature-major, AllGather h across the 8 cores per layer.
"""

import sys

sys.path.insert(0, "/opt/trn_rl_repo")

import numpy as np

import concourse.bass as bass
import concourse.mybir as mybir
import concourse.tile as tile_mod
from concourse.bass_utils import run_bass_kernel_spmd
from concourse.masks import make_identity
from concourse.tile import TileContext
from concourse.vector_clock import ScopedClock, VectorClock

# ----------------------------------------------------------------------------
# Toolchain workarounds: this walrus build rejects instructions carrying more
# than a couple of sync waits ("Too many sync wait commands").  Split waits
# onto dedicated same-engine NoOps.
# ----------------------------------------------------------------------------
WAIT_LIMIT = 1


def _patched_drain_and_barrier(self, tick_clock, wait_clock):
    nc = self.nc
    gc = tick_clock.global_clock
    n = len(gc)
    for i in range(n):
        t = gc[i]
        if t > 0:
            sub = [0] * n
            sub[i] = t
            nop_inst = nc.sync.nop(nofuse=True)
            wait_clock.add_sem_waits(nop_inst.ins, ScopedClock({None: VectorClock(sub)}))
    nc.sync.drain()
    nc.all_engine_barrier()
    popped = nc._tile_sem_poison_stack.pop()
    assert popped is self._sem_poison
    nc.clear_and_free_semaphores(list(self.sems.allocated().values()))
    nc.all_engine_barrier()


tile_mod.TileContext._drain_and_barrier = _patched_drain_and_barrier


def _split_excess_waits(nc, limit=WAIT_LIMIT):
    n_split = 0
    for _bbname, bbb in nc.bb_map.items():
        bb = bbb.bb
        insts = list(bb.instructions)
        out = []
        for inst in insts:
            si = inst.sync_info
            if si is not None and si.on_wait is not None and len(si.on_wait) > limit:
                waits = list(si.on_wait)
                movable = [w for w in waits if w.wait_reg is None]
                fixed = [w for w in waits if w.wait_reg is not None]
                nkeep_mov = max(0, limit - len(fixed))
                keep = fixed + (movable[len(movable) - nkeep_mov:] if nkeep_mov else [])
                extra = movable[: len(movable) - nkeep_mov]
                while extra:
                    chunk, extra = extra[:limit], extra[limit:]
                    nop = mybir.InstNoOp(name=nc.get_next_instruction_name())
                    nop.engine = inst.engine
                    nop.sync_info = mybir.SyncInfo(on_wait=chunk, on_update=[])
                    nop.bass_nofuse = True
                    nc.register_instruction(nop, overwrite=True)
                    out.append(nop)
                    n_split += 1
                inst.sync_info = mybir.SyncInfo(
                    on_wait=keep, on_update=list(si.on_update or [])
                )
            out.append(inst)
        bb.instructions = out
    return n_split


import concourse.bass_utils as _bu

_orig_run_command = _bu.run_command


def _run_command_no_birverifier(cmd, **kw):
    cmd = [
        c.replace("birverifier,", "") if isinstance(c, str) else c for c in cmd
    ]
    if _os.environ.get("LDWOPT", "0") == "1":
        cmd = [
            c.replace("--enable-ldw-opt=false", "--enable-ldw-opt=true")
            if isinstance(c, str)
            else c
            for c in cmd
        ]
    return _orig_run_command(cmd, **kw)


_bu.run_command = _run_command_no_birverifier


def _round_f32r(x):
    """Host replica of the fp32r rounding (RNE, 11 mantissa bits kept).

    PE fp32r operands must contain rounded bits: feeding raw fp32 bits makes
    the PE fault (verified on HW), so anything DMA'd into an fp32r tile is
    pre-rounded here.
    """
    ai = np.ascontiguousarray(x, np.float32).view(np.uint32)
    drop = np.uint32(12)
    half = np.uint32(1 << 11)
    lsb = ((ai >> drop) & np.uint32(1)).astype(np.uint32)
    out = ((ai + (half - np.uint32(1)) + lsb) >> drop) << drop
    return out.view(np.float32)


# ----------------------------------------------------------------------------
# Problem constants (hardcoded from the model definition)
# ----------------------------------------------------------------------------
N_NODES = 4096
N_EDGES = 32768
WIDTH = 64
KER_W = 1024
DEPTH = 4
KER_IN = 6
IN_W = 6
NCORES = 8
NLOC = N_NODES // NCORES  # 512 nodes per core
P = 128

_dt = mybir.dt
F32 = _dt.float32
F32R = _dt.float32r
BF16 = _dt.bfloat16
I32 = _dt.int32
ALU = mybir.AluOpType
AF = mybir.ActivationFunctionType
import os as _os
NACCS = int(_os.environ.get("MATVEC_ACCS", "8"))
ABLATE = _os.environ.get("ABLATE", "")
WPSCOPY = _os.environ.get("WPSCOPY", "") == "1"
SGP = _os.environ.get("S_GPSIMD", "0") == "1"


def _packf_layout(EP):
    """Flat f32 pack: offsets for every f32 input segment (srcf holds int
    indices as float values, converted on device)."""
    segs = [
        ("h0g", N_NODES * WIDTH),
        ("hfm0", WIDTH * NLOC),
        ("kb2s", DEPTH * P * (KER_W // P)),
        ("roota", DEPTH * (WIDTH + 1) * WIDTH),
        ("fc2a", WIDTH + 1),
        ("iota", P * NLOC),
        ("srcf", EP),
        ("dstl", EP),
        ("invde", EP),
    ]
    off, o = {}, 0
    for n, sz in segs:
        off[n] = o
        o += sz
    return off, o


def _packb_layout(EP):
    segs = [
        ("kw1a", DEPTH * (IN_W + 1) * (KER_W // 2)),
        ("kw2", DEPTH * (KER_W // 2) * KER_W),
        ("kw3", DEPTH * KER_W * WIDTH * WIDTH),
        ("eaT", (IN_W + 1) * EP),
    ]
    off, o = {}, 0
    for n, sz in segs:
        off[n] = o
        o += sz
    return off, o


def _build_nc(T, kb3_nonzero):
    """Build the SPMD Bass program for T 128-edge tiles per core."""
    B = (T + 3) // 4  # blocks of 512 edges (last may be ragged)
    EP = B * 512
    nc = bass.Bass("TRN2", target_bir_lowering=False, debug=False, num_devices=NCORES)

    # ---- dram inputs packed into two tensors: per-exec PJRT dispatch costs
    # ~56us per input tensor, so 13 inputs -> 2 saves ~0.6ms of wall time ----
    off_f, FTOT = _packf_layout(EP)
    off_b, BTOT = _packb_layout(EP)
    d_packf = nc.dram_tensor("packf", [FTOT], F32, kind="ExternalInput")
    d_packb = nc.dram_tensor("packb", [BTOT], BF16, kind="ExternalInput")
    d_kb3 = None
    if kb3_nonzero:
        d_kb3 = nc.dram_tensor("kb3r", [DEPTH, WIDTH, WIDTH], F32, kind="ExternalInput")

    def _f(name, ln, pat=None, extra=0, **kw):
        o = off_f[name] + extra
        ap = d_packf.ap()[o : o + ln]
        return ap.rearrange(pat, **kw) if pat else ap

    def _b(name, extra, ln, pat=None, **kw):
        o = off_b[name] + extra
        ap = d_packb.ap()[o : o + ln]
        return ap.rearrange(pat, **kw) if pat else ap

    d_out = nc.dram_tensor("out_loc", [1, NLOC], F32, kind="ExternalOutput")

    # ---- internal dram ----
    d_hloc = nc.dram_tensor("hloc", [NLOC, WIDTH], F32)
    d_hgat = [
        nc.dram_tensor(f"hgat{k}", [N_NODES, WIDTH], F32, addr_space="Shared")
        for k in range(DEPTH - 1)
    ]

    rg = [list(range(NCORES))]
    KC3 = KER_W // P  # 8 contraction chunks for kw3
    NC3 = WIDTH * WIDTH  # 4096 output cols

    with TileContext(nc) as tc:
        with (
            tc.tile_pool(name="pers", bufs=1) as pers,
            tc.tile_pool(name="wk", bufs=2) as wk,
            tc.tile_pool(name="ppw", bufs=2, space="PSUM") as ppw,
            tc.tile_pool(name="ppe", bufs=2, space="PSUM") as ppe,
            tc.tile_pool(name="ppm", bufs=1, space="PSUM") as ppm,
            tc.tile_pool(name="ppa", bufs=1, space="PSUM") as ppa,
        ):
            # ---------------- persistent tiles ----------------
            iota_s = pers.tile([P, NLOC], F32)
            nc.sync.dma_start(out=iota_s[:], in_=_f("iota", P * NLOC, "(p c) -> p c", p=P))
            ident = pers.tile([P, P], F32)
            make_identity(nc, ident[:])

            # per-edge metadata as [128, 4B] (covers T used tiles); src comes
            # in as float values and is converted to int32 on device
            srcF = pers.tile([P, 4 * B], F32)
            srcT = pers.tile([P, 4 * B], I32)
            dstT = pers.tile([P, 4 * B], F32)
            invT = pers.tile([P, 4 * B], F32)
            for (dsttile, nm) in ((srcF, "srcf"), (dstT, "dstl"), (invT, "invde")):
                nc.sync.dma_start(
                    out=dsttile[:], in_=_f(nm, EP, "(t p) -> p t", p=P)
                )
            nc.vector.tensor_copy(out=srcT[:], in_=srcF[:])

            # h feature-major augmented [65, 512]; row 64 = ones
            # (hfm0 pre-rounded on host; device relu copies re-round later)
            hfm = pers.tile([WIDTH + 1, NLOC], F32R)
            nc.sync.dma_start(
                out=hfm[0:WIDTH, :].bitcast(F32),
                in_=_f("hfm0", WIDTH * NLOC, "(w n) -> w n", w=WIDTH),
            )
            nc.vector.memset(hfm[WIDTH : WIDTH + 1, :].bitcast(F32), 1.0)

            fc2r = pers.tile([WIDTH + 1, 1], F32R)
            nc.sync.dma_start(
                out=fc2r[:].bitcast(F32),
                in_=_f("fc2a", WIDTH + 1, "(a b) -> a b", b=1),
            )

            # per-layer weight tiles (persistent slots, reloaded per layer;
            # per-chunk tiles so reloads only WAR-wait on their own chunk)
            # double-buffered per-layer weight slots (k%2): layer k+1's
            # weight DMAs have no WAR hazard on layer k's readers, so they
            # overlap layer k's compute
            kw1r_s = [
                pers.tile([IN_W + 1, KER_W // 2], BF16, name=f"kw1r{s}")
                for s in range(2)
            ]
            kw2rc_s = [
                [
                    pers.tile([P, KER_W], BF16, name=f"kw2rc{s}_{c}")
                    for c in range(KER_W // 2 // P)
                ]
                for s in range(2)
            ]
            kw3rc_s = [
                [pers.tile([P, NC3], BF16, name=f"kw3rc{s}_{c}") for c in range(KC3)]
                for s in range(2)
            ]
            rootr_s = [
                pers.tile([WIDTH + 1, WIDTH], F32R, name=f"rootr{s}")
                for s in range(2)
            ]
            kb2t_s = [
                pers.tile([P, KER_W // P], F32, name=f"kb2t{s}") for s in range(2)
            ]
            kb3t = pers.tile([WIDTH, WIDTH], F32R) if kb3_nonzero else None

            # working tiles allocated per use from `wk`
            e1r = pers.tile([P, 4 * 512], BF16)
            e2r = pers.tile([P, 8 * 512], BF16)

            for k in range(DEPTH):
                s_w = 0 if ABLATE == "reuseweights" else k % 2
                kw1r, kw2rc, kw3rc = kw1r_s[s_w], kw2rc_s[s_w], kw3rc_s[s_w]
                rootr, kb2t = rootr_s[s_w], kb2t_s[s_w]
                # ------------ load layer weights (pre-rounded on host) ------------
                if ABLATE != "reuseweights" or k == 0:
                    nc.sync.dma_start(
                        out=kw1r[:],
                        in_=_b("kw1a", k * (IN_W + 1) * (KER_W // 2),
                               (IN_W + 1) * (KER_W // 2), "(a b) -> a b", a=IN_W + 1),
                    )
                    for c in range(KER_W // 2 // P):
                        nc.sync.dma_start(
                            out=kw2rc[c][:],
                            in_=_b("kw2", (k * (KER_W // 2) + c * P) * KER_W,
                                   P * KER_W, "(p c) -> p c", p=P),
                        )
                    for kc in range(KC3):
                        nc.sync.dma_start(
                            out=kw3rc[kc][:],
                            in_=_b("kw3", (k * KER_W + kc * P) * NC3,
                                   P * NC3, "(p c) -> p c", p=P),
                        )
                    nc.sync.dma_start(
                        out=rootr[:].bitcast(F32),
                        in_=_f("roota", (WIDTH + 1) * WIDTH, "(a b) -> a b",
                               extra=k * (WIDTH + 1) * WIDTH, a=WIDTH + 1),
                    )
                    nc.sync.dma_start(
                        out=kb2t[:],
                        in_=_f("kb2s", P * (KER_W // P), "(p c) -> p c",
                               extra=k * P * (KER_W // P), p=P),
                    )
                if kb3_nonzero:
                    nc.sync.dma_start(out=kb3t[:].bitcast(F32), in_=d_kb3[k])

                aggP = ppa.tile([WIDTH, NLOC], F32, tag="aggP")
                htab = (
                    _f("h0g", N_NODES * WIDTH, "(n w) -> n w", n=N_NODES)
                    if k == 0
                    else d_hgat[k - 1][:]
                )
                pend_scatter = []

                def _emit_scatter(entries, aggP=aggP):
                    for (t, msgr) in entries:
                        S = wk.tile(
                            [P, NLOC], F32R, tag=f"S{t % 2}", bufs=2, name=f"S_{t}"
                        )
                        if ABLATE != "noscatter":
                            _seng = nc.gpsimd if SGP else nc.vector
                            _seng.tensor_scalar(
                                out=S[:], in0=iota_s[:], scalar1=dstT[:, t : t + 1],
                                scalar2=invT[:, t : t + 1], op0=ALU.is_equal,
                                op1=ALU.mult,
                            )
                            nc.tensor.matmul(
                                out=aggP[:], lhsT=msgr[:], rhs=S[:],
                                start=(t == 0), stop=False, skip_group_check=True,
                            )
                        elif t == 0:
                            nc.vector.memset(S[:].bitcast(F32), 0.0)
                            nc.tensor.matmul(
                                out=aggP[:], lhsT=msgr[:], rhs=S[:],
                                start=True, stop=False, skip_group_check=True,
                            )

                for blk in range(B):
                    eoff = blk * 512
                    # ---- this block's edge attrs [7, 512] ----
                    ea_rb = wk.tile([IN_W + 1, 512], BF16, tag="ea_rb")
                    nc.sync.dma_start(
                        out=ea_rb[:],
                        in_=_b("eaT", 0, (IN_W + 1) * EP, "(a e) -> a e",
                               a=IN_W + 1)[:, eoff : eoff + 512],
                    )
                    # ---- e1 = relu(ea @ kw1_aug) : [512 feats, 512 edges] ----
                    for mc in range(4) if ABLATE != "nomlp" else []:
                        pe1 = ppe.tile([P, 512], F32, tag="pe")
                        nc.tensor.matmul(
                            out=pe1[:],
                            lhsT=kw1r[:, mc * P : (mc + 1) * P],
                            rhs=ea_rb[:],
                            start=True,
                            stop=True,
                        )
                        nc.scalar.activation(
                            e1r[:, mc * 512 : (mc + 1) * 512], pe1[:], AF.Relu
                        )
                    # ---- e2 = relu(e1 @ kw2 + kb2) : [1024 feats, 512 edges] ----
                    for mc2 in range(8) if ABLATE != "nomlp" else []:
                        pe2 = ppe.tile([P, 512], F32, tag="pe")
                        for kc in range(4):
                            nc.tensor.matmul(
                                out=pe2[:],
                                lhsT=kw2rc[kc][:, mc2 * P : (mc2 + 1) * P],
                                rhs=e1r[:, kc * 512 : (kc + 1) * 512],
                                start=(kc == 0),
                                stop=(kc == 3),
                            )
                        nc.scalar.activation(
                            e2r[:, mc2 * 512 : (mc2 + 1) * 512],
                            pe2[:],
                            AF.Relu,
                            bias=kb2t[:, mc2 : mc2 + 1],
                        )
                    # ---- per 128-edge tile (ragged last block) ----
                    nt_blk = min(4, T - blk * 4)
                    msgrs = []
                    for t4 in range(nt_blk):
                        t = blk * 4 + t4
                        hsrc = wk.tile([P, WIDTH], F32, tag="hsrc", bufs=8)
                        if ABLATE != "nogather":
                            nc.gpsimd.indirect_dma_start(
                                out=hsrc[:],
                                out_offset=None,
                                in_=htab,
                                in_offset=bass.IndirectOffsetOnAxis(
                                    ap=srcT[:, t : t + 1], axis=0
                                ),
                            )
                        else:
                            nc.vector.memset(hsrc[:], 0.01)
                        # 8 independent accumulators keep the DVE pipeline full
                        # (serial FMA chain measured 286 ns/op vs 88 ns with 8)
                        accs = [
                            wk.tile([P, WIDTH], F32, name=f"macc{j}_{t}", tag=f"macc{j}", bufs=2)
                            for j in range(NACCS)
                        ]
                        msgr = wk.tile([P, WIDTH], F32R, tag=f"msgr{t4}", bufs=2)
                        tcor = None
                        if kb3_nonzero:
                            tps = ppm.tile([WIDTH, P], F32, tag="tp")
                            nc.tensor.transpose(out=tps[:], in_=hsrc[:], identity=ident[:])
                            hsT = wk.tile([WIDTH, P], F32R, tag="hsT")
                            nc.scalar.activation(hsT[:], tps[:], AF.Copy)
                            tcor = ppm.tile([P, WIDTH], F32, tag="tc")
                            nc.tensor.matmul(
                                out=tcor[:], lhsT=hsT[:], rhs=kb3t[:], start=True, stop=True
                            )
                        # kc-outer over 4 PSUM banks so consecutive matmuls
                        # share the same stationary (walrus ldw-opt dedupes
                        # the reloads)
                        for half in range(2):
                            wpsb = [
                                ppw.tile(
                                    [P, 512], F32, tag=f"wps{c}", bufs=1,
                                    name=f"wpsb{c}_{t}_{half}",
                                )
                                for c in range(4)
                            ]
                            if ABLATE == "nowe":
                                for c in range(4):
                                    nc.vector.memset(wpsb[c][:], 0.01)
                            else:
                                for kc in range(KC3):
                                    for c in range(4):
                                        cc = half * 4 + c
                                        nc.tensor.matmul(
                                            out=wpsb[c][:],
                                            lhsT=e2r[:, kc * 512 + t4 * P : kc * 512 + (t4 + 1) * P],
                                            rhs=kw3rc[kc][:, cc * 512 : (cc + 1) * 512],
                                            start=(kc == 0),
                                            stop=(kc == KC3 - 1),
                                        )
                            for c in range(4):
                                cc = half * 4 + c
                                rdsrc = wpsb[c]
                                for j in range((0 if ABLATE == "nomatvec" else 8)):
                                    i_ = cc * 8 + j
                                    sl = rdsrc[:, j * WIDTH : (j + 1) * WIDTH]
                                    sc = hsrc[:, i_ : i_ + 1]
                                    ja = j % NACCS
                                    if cc * 8 + j < NACCS:
                                        nc.vector.tensor_scalar(
                                            out=accs[ja][:], in0=sl, scalar1=sc,
                                            scalar2=None, op0=ALU.mult,
                                        )
                                    else:
                                        nc.vector.scalar_tensor_tensor(
                                            out=accs[ja][:], in0=sl, scalar=sc,
                                            in1=accs[ja][:], op0=ALU.mult, op1=ALU.add,
                                        )
                        if ABLATE == "nomatvec":
                            for j in range(NACCS):
                                nc.vector.memset(accs[j][:], 0.0)
                        # tree-reduce the accumulators
                        stride = 1
                        while stride < NACCS:
                            for d in range(0, NACCS, 2 * stride):
                                if d + stride < NACCS and not (
                                    stride * 2 >= NACCS and d == 0 and not kb3_nonzero
                                ):
                                    nc.vector.tensor_add(
                                        out=accs[d][:], in0=accs[d][:], in1=accs[d + stride][:]
                                    )
                            stride *= 2
                        if kb3_nonzero:
                            nc.vector.tensor_add(out=accs[0][:], in0=accs[0][:], in1=tcor[:])
                            nc.vector.tensor_copy(out=msgr[:], in_=accs[0][:])
                        elif NACCS > 1:
                            nc.vector.tensor_add(
                                out=msgr[:], in0=accs[0][:], in1=accs[NACCS // 2][:]
                            )
                        else:
                            nc.vector.tensor_copy(out=msgr[:], in_=accs[0][:])
                        msgrs.append((t, msgr))
                    # ---- scatter phase for the PREVIOUS block: deferring by
                    # one block gives DVE a full block of slack, so the PE
                    # never stalls waiting for the last tile's matvec ----
                    if pend_scatter:
                        _emit_scatter(pend_scatter)
                    pend_scatter = msgrs
                if pend_scatter:
                    _emit_scatter(pend_scatter)
                    pend_scatter = []
                # ---- update: h = relu(agg*inv_deg(folded) + h@root + bias) ----
                nc.tensor.matmul(
                    out=aggP[:], lhsT=rootr[:], rhs=hfm[:],
                    start=False, stop=True, skip_group_check=True,
                )
                hnf = wk.tile([WIDTH, NLOC], F32, tag="hnf")
                nc.scalar.activation(hnf[:], aggP[:], AF.Relu)
                nc.scalar.activation(hfm[0:WIDTH, :], hnf[:], AF.Copy)
                if k < DEPTH - 1:
                    for c in range(NLOC // P):
                        tp = ppm.tile([P, WIDTH], F32, tag="tp")
                        nc.tensor.transpose(
                            out=tp[:],
                            in_=hnf[:, c * P : (c + 1) * P],
                            identity=ident[0:WIDTH, 0:WIDTH],
                        )
                        hts = wk.tile([P, WIDTH], F32, tag="hts")
                        nc.vector.tensor_copy(out=hts[:], in_=tp[:])
                        nc.sync.dma_start(out=d_hloc[c * P : (c + 1) * P, :], in_=hts[:])
                    if ABLATE != "nocoll":
                        nc.gpsimd.collective_compute(
                            "AllGather",
                            ALU.bypass,
                            ins=[d_hloc[:]],
                            outs=[d_hgat[k][:]],
                            replica_groups=rg,
                        )
                    else:
                        nc.sync.dma_start(
                            out=d_hgat[k][0:NLOC, :], in_=d_hloc[:]
                        )
            # ---- readout: out = h @ fc2 + b ----
            pf = ppm.tile([1, NLOC], F32, tag="tp")
            nc.tensor.matmul(out=pf[:], lhsT=fc2r[:], rhs=hfm[:], start=True, stop=True)
            ot = wk.tile([1, NLOC], F32, tag="hnf")
            nc.vector.tensor_copy(out=ot[:], in_=pf[:])
            nc.sync.dma_start(out=d_out[:], in_=ot[:])

    _split_excess_waits(nc)
    return nc


def _host_prep(x, edge_attr, fc1_w, fc1_b, kw1, kb1, kw2, kb2, kw3, kb3,
               root, bias, fc2_w, fc2_b, edge_index):
    f = np.float32
    x = np.asarray(x, f)
    edge_attr = np.asarray(edge_attr, f)
    fc1_w = np.asarray(fc1_w, f); fc1_b = np.asarray(fc1_b, f)
    kw1 = np.asarray(kw1, f); kb1 = np.asarray(kb1, f)
    kw2 = np.asarray(kw2, f); kb2 = np.asarray(kb2, f)
    kw3 = np.asarray(kw3, f); kb3 = np.asarray(kb3, f)
    root = np.asarray(root, f); bias = np.asarray(bias, f)
    fc2_w = np.asarray(fc2_w, f); fc2_b = np.asarray(fc2_b, f)
    ei = np.asarray(edge_index)
    src = ei[0].astype(np.int64)
    dst = ei[1].astype(np.int64)

    deg = np.bincount(dst, minlength=N_NODES).astype(f)
    inv_deg = np.zeros(N_NODES, f)
    np.divide(f(1.0), deg, out=inv_deg, where=deg > 0)

    order = np.argsort(dst, kind="stable")
    dsts = dst[order]
    bounds = np.searchsorted(dsts, np.arange(0, N_NODES + 1, NLOC))
    counts = np.diff(bounds)
    T = int(np.ceil(counts.max() / 128.0))
    EP = ((T + 3) // 4) * 512

    h0 = (x @ fc1_w + fc1_b).astype(f)

    import ml_dtypes

    bf = ml_dtypes.bfloat16
    kw1_aug = np.concatenate([kw1, kb1[:, None, :]], axis=1).astype(bf)
    kw2 = np.ascontiguousarray(kw2).astype(bf)
    kw3 = np.ascontiguousarray(kw3).astype(bf)
    kb2s = np.stack([kb2[k].reshape(KER_W // P, P).T for k in range(DEPTH)]).astype(f)
    root_aug = _round_f32r(np.concatenate([root, bias[:, None, :]], axis=1))
    fc2_aug = _round_f32r(np.concatenate([fc2_w, fc2_b.reshape(1, 1)], axis=0))
    iota = np.tile(np.arange(NLOC, dtype=f)[None, :], (P, 1))
    kb3_nonzero = bool(np.any(kb3))
    kb3r = _round_f32r(kb3.reshape(DEPTH, WIDTH, WIDTH))

    packb_w = np.concatenate(
        [
            np.asarray(kw1_aug, bf).ravel(),
            np.asarray(kw2, bf).ravel(),
            np.asarray(kw3, bf).ravel(),
        ]
    )
    in_maps = []
    for m in range(NCORES):
        sel = order[bounds[m] : bounds[m + 1]]
        cnt = len(sel)
        eaT = np.zeros((IN_W + 1, EP), f)
        eaT[0:IN_W, :cnt] = edge_attr[sel].T
        eaT[IN_W, :cnt] = 1.0
        srcf = np.zeros(EP, f)
        srcf[:cnt] = src[sel].astype(f)
        dstl = np.full(EP, -1.0, f)
        dstl[:cnt] = (dst[sel] - NLOC * m).astype(f)
        invde = np.zeros(EP, f)
        invde[:cnt] = inv_deg[dst[sel]]
        hfm0_m = _round_f32r(np.ascontiguousarray(h0[NLOC * m : NLOC * (m + 1)].T))
        packf = np.concatenate(
            [
                h0.ravel(),
                hfm0_m.ravel(),
                kb2s.ravel(),
                root_aug.ravel(),
                fc2_aug.ravel(),
                iota.ravel(),
                srcf,
                dstl,
                invde,
            ]
        ).astype(f)
        packb = np.concatenate([packb_w, eaT.astype(bf).ravel()])
        im = {"packf": packf, "packb": packb}
        if kb3_nonzero:
            im["kb3r"] = kb3r
        in_maps.append(im)
    return in_maps, T, kb3_nonzero


_BUILD_CACHE = {}


def kernel(**inputs) -> np.ndarray:
    in_maps, T, kb3_nonzero = _host_prep(**inputs)
    key = (T, kb3_nonzero)
    if key not in _BUILD_CACHE:
        _BUILD_CACHE[key] = _build_nc(T, kb3_nonzero)
    nc = _BUILD_CACHE[key]
    res = run_bass_kernel_spmd(nc, in_maps, list(range(NCORES)))
    out = np.concatenate(
        [res.results[m]["out_loc"].reshape(NLOC, 1) for m in range(NCORES)], axis=0
    )
    return out.astype(np.float32)

